# revision 16
# baseline (speedup 1.0000x reference)
"""Trainium2 Bass kernel for CausalSelfAttentionARMA (eval forward).

Sharding: 8 cores = 2 batches x 4 head-groups (4 heads each, d=64, group
cols=256). Each core computes its (batch, head-group) shard end-to-end and
returns a transposed partial output [C, T]; the host sums partials per batch
and adds 2*b_proj.

Device algorithm per core (all matmuls float32r = full-rate tf32-like):
  - projections qT/kT/k2T = W.T @ x.T streamed by 512-col slices of T
  - causal attention with transposed scores: sT[kj,qi] = kT_j.T @ qT_g,
    exp on ACT (scale=1/8 folded in), AV via v-stationary matmul whose
    ones-column also produces the softmax denominators
  - MA pass: e = vshift - y (y transposed back via PE), q shifted one col
  - out projection transposed: outT[cblock] = Wp_cb.T @ zT
"""
import numpy as np

import concourse.bass as bass
import concourse.bacc as bacc
import concourse.tile as tile
from concourse import mybir
from concourse.bass_utils import run_bass_kernel_spmd

F32 = mybir.dt.float32
F32R = mybir.dt.float32r

B, T, C = 2, 2048, 1024
H, D = 16, 64
HPC = 4                # heads per core
DG = HPC * D           # 256
NT = T // 128          # 16 t-tiles
NG = 4                 # q groups of 512
CHUNK = 2              # kj blocks per exp chunk (psum banks per scores tile)

_BUILT = None


def _chunks(lst, n):
    return [lst[i:i + n] for i in range(0, len(lst), n)]


def _build():
    nc = bacc.Bacc("TRN2", target_bir_lowering=False, debug=False, num_devices=8)

    xT = nc.declare_dram_parameter("xT", [C, T], F32, isOutput=False)
    vaug = nc.declare_dram_parameter("vaug", [HPC * 128, NT * 65], F32, isOutput=False)
    vsh = nc.declare_dram_parameter("vsh", [128, NT * DG], F32, isOutput=False)
    wq = nc.declare_dram_parameter("wq", [128, 8 * DG], F32, isOutput=False)
    wk = nc.declare_dram_parameter("wk", [128, 8 * DG], F32, isOutput=False)
    wk2 = nc.declare_dram_parameter("wk2", [128, 8 * DG], F32, isOutput=False)
    wp = nc.declare_dram_parameter("wp", [128, 2 * C], F32, isOutput=False)
    bq = nc.declare_dram_parameter("bq", [128, 2], F32, isOutput=False)
    bk = nc.declare_dram_parameter("bk", [128, 2], F32, isOutput=False)
    bk2 = nc.declare_dram_parameter("bk2", [128, 2], F32, isOutput=False)
    maskp = nc.declare_dram_parameter("mask", [128, 128], F32, isOutput=False)
    identp = nc.declare_dram_parameter("ident", [128, 64], F32, isOutput=False)
    onesp = nc.declare_dram_parameter("ones16", [128, 16, 1], F32, isOutput=False)
    outT = nc.declare_dram_parameter("outT", [C, T], F32, isOutput=True)

    with tile.TileContext(nc) as tc:
        import contextlib
        with contextlib.ExitStack() as ctx:
            const = ctx.enter_context(tc.tile_pool(name="const", bufs=1))
            persist = ctx.enter_context(tc.tile_pool(name="persist", bufs=1))
            small = ctx.enter_context(tc.tile_pool(name="small", bufs=3))

            # ---- constants / persistent inputs ----
            mask_sb = const.tile([128, 128], F32, tag="mask")
            nc.sync.dma_start(mask_sb[:], maskp[:])
            ident_sb = const.tile([128, 64], F32R, tag="ident")
            nc.sync.dma_start(ident_sb[:], identp[:].bitcast(F32R))
            wp_sb = const.tile([128, 2 * C], F32R, tag="wp")
            nc.sync.dma_start(wp_sb[:], wp[:].bitcast(F32R))
            bias_sb = {}
            for nm, par in (("bq", bq), ("bk", bk), ("bk2", bk2)):
                t = const.tile([128, 2], F32, tag=nm, name=nm)
                nc.sync.dma_start(t[:], par[:])
                bias_sb[nm] = t

            vaug_sb = []
            for h in range(HPC):
                t = persist.tile([128, NT * 65], F32R, tag=f"vaug{h}", name=f"vaug{h}")
                nc.sync.dma_start(t[:], vaug[h * 128:(h + 1) * 128, :].bitcast(F32R))
                vaug_sb.append(t)
            vsh_sb = persist.tile([128, NT * DG], F32, tag="vsh")
            nc.sync.dma_start(vsh_sb[:], vsh[:])

            qT = [persist.tile([128, T + 8], F32R, tag=f"qT{p}", name=f"qT{p}") for p in range(2)]
            kT = [persist.tile([128, T], F32R, tag=f"kT{p}", name=f"kT{p}") for p in range(2)]
            k2T = [persist.tile([128, T], F32R, tag=f"k2T{p}", name=f"k2T{p}") for p in range(2)]
            zT = [persist.tile([128, T], F32R, tag=f"zT{p}", name=f"zT{p}") for p in range(2)]
            eaug_sb = [persist.tile([128, NT * 65], F32R, tag=f"eaug{h}", name=f"eaug{h}")
                       for h in range(HPC)]

            # ---- projections:  dst[p][:, n*512:+512] = W_chunk.T @ xT + b ----
            projs = []   # (w_param, dst_list, bias_tile)
            with tc.tile_pool(name="wpool", bufs=1) as wpool, \
                 tc.tile_pool(name="xs", bufs=4) as xs, \
                 tc.tile_pool(name="pproj", bufs=1, space="PSUM") as pproj:
                for nm, par in (("wq", wq), ("wk", wk), ("wk2", wk2)):
                    t = wpool.tile([128, 8 * DG], F32R, tag=nm, name=nm)
                    nc.sync.dma_start(t[:], par[:].bitcast(F32R))
                    projs.append(t)
                wq_sb, wk_sb, wk2_sb = projs

                plan = [(wq_sb, qT, "bq"), (wk_sb, kT, "bk"), (wk2_sb, k2T, "bk2")]
                for n in range(4):
                    accs = []
                    for c in range(8):
                        xp = xs.tile([128, 512], F32R, tag="xp")
                        nc.sync.dma_start(
                            xp[:], xT[c * 128:(c + 1) * 128,
                                      n * 512:(n + 1) * 512].bitcast(F32R))
                        for pi, (w_sb, dsts, bnm) in enumerate(plan):
                            for p in range(2):
                                if c == 0:
                                    accs.append(pproj.tile([128, 512], F32,
                                                           tag=f"acc{pi}{p}", name=f"acc{pi}{p}"))
                                nc.tensor.matmul(
                                    accs[pi * 2 + p][:],
                                    w_sb[:, c * 256 + p * 128: c * 256 + p * 128 + 128],
                                    xp[:],
                                    start=(c == 0), stop=(c == 7))
                    for pi, (w_sb, dsts, bnm) in enumerate(plan):
                        for p in range(2):
                            nc.vector.tensor_scalar_add(
                                dsts[p][:, n * 512:(n + 1) * 512],
                                accs[pi * 2 + p][:],
                                bias_sb[bnm][:, p:p + 1])

            # pad col for shifted q (col T = col T-1; value discarded)
            for p in range(2):
                nc.vector.tensor_copy(qT[p][:, T:T + 1], qT[p][:, T - 1:T])

            # ---- attention helper ----
            actx = contextlib.ExitStack()
            ps_pool = actx.enter_context(
                tc.tile_pool(name="ps", bufs=2, space="PSUM"))
            py_pool = actx.enter_context(
                tc.tile_pool(name="py", bufs=4, space="PSUM"))
            expp = actx.enter_context(tc.tile_pool(name="expp", bufs=3))

            deferred = []   # (fn) finalize closures, flushed 2 groups late

            def flush_deferred(keep=2):
                while len(deferred) > keep:
                    deferred.pop(0)()

            def attn_pass(h, keysT, statT, qoff, z_write):
                """One causal attention pass for head h.
                keysT: [128, T] pair tile with keys for the pair (row0 offset);
                statT: [128, NT*65] stationary (values+ones);
                qoff: extra column offset into qT (0 or 1);
                z_write(g, py): finalize group's normalized output."""
                p, row0 = h // 2, (h % 2) * 64
                for g in range(NG):
                    flush_deferred(keep=2)
                    py = py_pool.tile([65, 512], F32, tag="py")
                    jlist = list(range(4 * g + 4))
                    for chunk in _chunks(jlist, CHUNK):
                        ps = ps_pool.tile([128, 512 * CHUNK], F32, tag="ps")
                        for idx, j in enumerate(chunk):
                            col0 = max(0, (j - 4 * g) * 128)
                            nc.tensor.matmul(
                                ps[:, idx * 512 + col0:(idx + 1) * 512],
                                keysT[row0:row0 + 64, j * 128:(j + 1) * 128],
                                qT[p][row0:row0 + 64,
                                      qoff + g * 512 + col0: qoff + (g + 1) * 512],
                                start=True, stop=True)
                            if j >= 4 * g:
                                nc.vector.tensor_add(
                                    ps[:, idx * 512 + col0: idx * 512 + col0 + 128],
                                    ps[:, idx * 512 + col0: idx * 512 + col0 + 128],
                                    mask_sb[:])
                        texp = expp.tile([128, 512 * CHUNK], F32R, tag="exp")
                        w = 512 * len(chunk)
                        nc.scalar.activation(texp[:, 0:w], ps[:, 0:w],
                                             mybir.ActivationFunctionType.Exp,
                                             scale=0.125)
                        for idx, j in enumerate(chunk):
                            col0 = max(0, (j - 4 * g) * 128)
                            nc.tensor.matmul(
                                py[:, col0:512],
                                statT[:, j * 65:(j + 1) * 65],
                                texp[:, idx * 512 + col0:(idx + 1) * 512],
                                start=(j == 0), stop=(j == jlist[-1]),
                                skip_group_check=True)
                    deferred.append(lambda g=g, py=py: z_write(g, py))

            # ---- pass 1 (AR) ----
            def run_pass(h, keysT, statT, qoff, fin):
                def zw(g, py):
                    rec = small.tile([1, 512], F32, tag="rec", name="rec")
                    nc.vector.reciprocal(rec[:], py[64:65, :])
                    rbc = small.tile([64, 512], F32, tag="rbc", name="rbc")
                    nc.gpsimd.partition_broadcast(rbc[:], rec[:])
                    fin(g, py, rbc)
                attn_pass(h, keysT, statT, qoff, zw)

            def z1_fin(h):
                p, row0 = h // 2, (h % 2) * 64

                def fin(g, py, rbc):
                    nc.vector.tensor_mul(
                        zT[p][row0:row0 + 64, g * 512:(g + 1) * 512],
                        py[0:64, :], rbc[:])
                return fin

            for h in range(HPC):
                run_pass(h, kT[h // 2], vaug_sb[h], 0, z1_fin(h))

            # ---- e = vshift - y  (into eaug, stationary layout) ----
            flush_deferred(keep=0)
            for h in range(HPC):
                p, row0 = h // 2, (h % 2) * 64
                for j in range(NT):
                    ptr = py_pool.tile([128, 64], F32R, tag="py")
                    nc.tensor.transpose(
                        ptr[:], zT[p][row0:row0 + 64, j * 128:(j + 1) * 128],
                        ident_sb[row0:row0 + 64, :])
                    nc.vector.tensor_sub(
                        eaug_sb[h][:, j * 65: j * 65 + 64],
                        vsh_sb[:, j * DG + h * 64: j * DG + h * 64 + 64],
                        ptr[:])
                ones_ap = eaug_sb[h][:].rearrange(
                    "p (j c) -> p j c", c=65)[:, :, 64:65]
                nc.sync.dma_start(ones_ap, onesp[:].bitcast(F32R))

            # ---- pass 2 (MA) ----
            def z2_fin(h):
                p, row0 = h // 2, (h % 2) * 64

                def fin(g, py, rbc):
                    tmp = small.tile([128, 512], F32, tag="tmp")
                    nc.vector.tensor_mul(tmp[row0:row0 + 64, :], py[0:64, :],
                                         rbc[:])
                    wd = 512 if g < NG - 1 else 511
                    dst = zT[p][row0:row0 + 64, g * 512 + 1: g * 512 + 1 + wd]
                    nc.vector.tensor_add(dst, dst, tmp[row0:row0 + 64, 0:wd])
                return fin

            for h in range(HPC):
                run_pass(h, k2T[h // 2], eaug_sb[h], 1, z2_fin(h))

            # ---- out projection: outT[cb] = sum_cc Wp[cc,cb].T @ zT[cc] ----
            flush_deferred(keep=0)
            actx.close()
            with tc.tile_pool(name="po", bufs=2, space="PSUM") as po, \
                 tc.tile_pool(name="ost", bufs=2) as ost:
                for cb in range(8):
                    acc = po.tile([128, T], F32, tag="po")
                    for cc in range(2):
                        for n in range(4):
                            nc.tensor.matmul(
                                acc[:, n * 512:(n + 1) * 512],
                                wp_sb[:, cc * C + cb * 128: cc * C + cb * 128 + 128],
                                zT[cc][:, n * 512:(n + 1) * 512],
                                start=(cc == 0), stop=(cc == 1))
                    stg = ost.tile([128, T], F32, tag="stg", name="stg")
                    nc.scalar.activation(stg[:], acc[:],
                                         mybir.ActivationFunctionType.Copy)
                    nc.sync.dma_start(outT[cb * 128:(cb + 1) * 128, :], stg[:])

    nc.compile()
    return nc


def _get_built():
    global _BUILT
    if _BUILT is None:
        _BUILT = _build()
    return _BUILT


def _prep_core(x, W_attn, b_attn, W_k2, b_k2, W_proj, core):
    b, hg = core // 4, core % 4
    cs = hg * DG
    xb = np.asarray(x[b], dtype=np.float32)
    xT = np.ascontiguousarray(xb.T)
    xh = xb[:, cs:cs + DG]

    va = np.empty((HPC, NT, 128, 65), np.float32)
    for h in range(HPC):
        va[h, :, :, :64] = xh[:, h * D:(h + 1) * D].reshape(NT, 128, D)
        va[h, :, :, 64] = 1.0
    vaug = va.transpose(0, 2, 1, 3).reshape(HPC * 128, NT * 65)

    vs = np.concatenate([xh[1:], np.zeros((1, DG), np.float32)], axis=0)
    vsh = vs.reshape(NT, 128, DG).transpose(1, 0, 2).reshape(128, NT * DG)

    def wslice(Wfull, c0):
        return np.ascontiguousarray(
            Wfull[:, c0:c0 + DG].reshape(8, 128, DG).transpose(1, 0, 2)
            .reshape(128, 8 * DG))

    wq = wslice(W_attn, cs)
    wk = wslice(W_attn, C + cs)
    wk2 = wslice(W_k2, cs)
    wp = np.ascontiguousarray(
        W_proj[cs:cs + DG, :].reshape(2, 128, C).transpose(1, 0, 2)
        .reshape(128, 2 * C))

    bqv = np.ascontiguousarray(b_attn[cs:cs + DG].reshape(2, 128).T)
    bkv = np.ascontiguousarray(b_attn[C + cs:C + cs + DG].reshape(2, 128).T)
    bk2v = np.ascontiguousarray(b_k2[cs:cs + DG].reshape(2, 128).T)

    return dict(xT=xT, vaug=vaug, vsh=vsh, wq=wq, wk=wk, wk2=wk2, wp=wp,
                bq=bqv, bk=bkv, bk2=bk2v)


def kernel(x, W_attn, b_attn, W_k2, b_k2, W_proj, b_proj):
    x = np.asarray(x, np.float32)
    W_attn = np.asarray(W_attn, np.float32)
    b_attn = np.asarray(b_attn, np.float32)
    W_k2 = np.asarray(W_k2, np.float32)
    b_k2 = np.asarray(b_k2, np.float32)
    W_proj = np.asarray(W_proj, np.float32)
    b_proj = np.asarray(b_proj, np.float32)

    mask = np.zeros((128, 128), np.float32)
    for kj in range(1, 128):
        mask[kj, :kj] = -1e5
    ident = np.concatenate([np.eye(64, dtype=np.float32)] * 2, axis=0)

    in_maps = []
    for core in range(8):
        m = _prep_core(x, W_attn, b_attn, W_k2, b_k2, W_proj, core)
        m["mask"] = mask
        m["ident"] = ident
        m["ones16"] = np.ones((128, 16, 1), np.float32)
        in_maps.append(m)

    nc = _get_built()
    res = run_bass_kernel_spmd(nc, in_maps, list(range(8)))

    out = np.zeros((B, T, C), np.float32)
    for core in range(8):
        out[core // 4] += res.results[core]["outT"].T
    out += 2.0 * b_proj
    return out


# revision 17
# speedup vs baseline: 1.0802x; 1.0802x over previous
"""Trainium2 Bass kernel for CausalSelfAttentionARMA (eval forward).

Sharding: 8 cores = 2 batches x 4 head-groups (4 heads each, d=64, group
cols=256). Each core computes its (batch, head-group) shard end-to-end and
returns a transposed partial output [C, T]; the host sums partials per batch
and adds 2*b_proj.

Device algorithm per core (all matmuls float32r = full-rate tf32-like):
  - projections qT/kT/k2T = W.T @ x.T streamed by 512-col slices of T
  - causal attention with transposed scores: sT[kj,qi] = kT_j.T @ qT_g,
    exp on ACT (scale=1/8 folded in), AV via v-stationary matmul whose
    ones-column also produces the softmax denominators
  - MA pass: e = vshift - y (y transposed back via PE), q shifted one col
  - out projection transposed: outT[cblock] = Wp_cb.T @ zT
"""
import numpy as np

import concourse.bass as bass
import concourse.bacc as bacc
import concourse.tile as tile
from concourse import mybir
from concourse.bass_utils import run_bass_kernel_spmd

F32 = mybir.dt.float32
F32R = mybir.dt.float32r
BF16 = mybir.dt.bfloat16

B, T, C = 2, 2048, 1024
H, D = 16, 64
HPC = 4                # heads per core
DG = HPC * D           # 256
NT = T // 128          # 16 t-tiles
NG = 4                 # q groups of 512
CHUNK = 2              # kj blocks per exp chunk (psum banks per scores tile)

_BUILT = None


def _chunks(lst, n):
    return [lst[i:i + n] for i in range(0, len(lst), n)]


def _build():
    nc = bacc.Bacc("TRN2", target_bir_lowering=False, debug=False, num_devices=8)

    xT = nc.declare_dram_parameter("xT", [C, T], F32, isOutput=False)
    vaug = nc.declare_dram_parameter("vaug", [HPC * 128, NT * 65], BF16, isOutput=False)
    vsh = nc.declare_dram_parameter("vsh", [128, NT * DG], F32, isOutput=False)
    wq = nc.declare_dram_parameter("wq", [128, 8 * DG], F32, isOutput=False)
    wk = nc.declare_dram_parameter("wk", [128, 8 * DG], F32, isOutput=False)
    wk2 = nc.declare_dram_parameter("wk2", [128, 8 * DG], F32, isOutput=False)
    wp = nc.declare_dram_parameter("wp", [128, 2 * C], F32, isOutput=False)
    bq = nc.declare_dram_parameter("bq", [128, 2], F32, isOutput=False)
    bk = nc.declare_dram_parameter("bk", [128, 2], F32, isOutput=False)
    bk2 = nc.declare_dram_parameter("bk2", [128, 2], F32, isOutput=False)
    maskp = nc.declare_dram_parameter("mask", [128, 128], F32, isOutput=False)
    identp = nc.declare_dram_parameter("ident", [128, 64], F32, isOutput=False)
    onesp = nc.declare_dram_parameter("ones16", [128, 16, 1], BF16, isOutput=False)
    outT = nc.declare_dram_parameter("outT", [C, T], F32, isOutput=True)

    with tile.TileContext(nc) as tc:
        import contextlib
        with contextlib.ExitStack() as ctx:
            const = ctx.enter_context(tc.tile_pool(name="const", bufs=1))
            persist = ctx.enter_context(tc.tile_pool(name="persist", bufs=1))
            small = ctx.enter_context(tc.tile_pool(name="small", bufs=3))

            # ---- constants / persistent inputs ----
            mask_sb = const.tile([128, 128], F32, tag="mask")
            nc.sync.dma_start(mask_sb[:], maskp[:])
            ident_sb = const.tile([128, 64], F32R, tag="ident")
            nc.sync.dma_start(ident_sb[:], identp[:].bitcast(F32R))
            wp_sb = const.tile([128, 2 * C], F32R, tag="wp")
            nc.sync.dma_start(wp_sb[:], wp[:].bitcast(F32R))
            bias_sb = {}
            for nm, par in (("bq", bq), ("bk", bk), ("bk2", bk2)):
                t = const.tile([128, 2], F32, tag=nm, name=nm)
                nc.sync.dma_start(t[:], par[:])
                bias_sb[nm] = t

            vaug_sb = []
            for h in range(HPC):
                t = persist.tile([128, NT * 65], BF16, tag=f"vaug{h}", name=f"vaug{h}")
                nc.sync.dma_start(t[:], vaug[h * 128:(h + 1) * 128, :])
                vaug_sb.append(t)
            vsh_sb = persist.tile([128, NT * DG], F32, tag="vsh")
            nc.sync.dma_start(vsh_sb[:], vsh[:])

            qT = [persist.tile([128, T + 8], BF16, tag=f"qT{p}", name=f"qT{p}") for p in range(2)]
            kT = [persist.tile([128, T], BF16, tag=f"kT{p}", name=f"kT{p}") for p in range(2)]
            k2T = [persist.tile([128, T], BF16, tag=f"k2T{p}", name=f"k2T{p}") for p in range(2)]
            zT = [persist.tile([128, T], F32R, tag=f"zT{p}", name=f"zT{p}") for p in range(2)]
            eaug_sb = [persist.tile([128, NT * 65], BF16, tag=f"eaug{h}", name=f"eaug{h}")
                       for h in range(HPC)]

            # ---- projections:  dst[p][:, n*512:+512] = W_chunk.T @ xT + b ----
            projs = []   # (w_param, dst_list, bias_tile)
            with tc.tile_pool(name="wpool", bufs=1) as wpool, \
                 tc.tile_pool(name="xs", bufs=4) as xs, \
                 tc.tile_pool(name="pproj", bufs=1, space="PSUM") as pproj:
                for nm, par in (("wq", wq), ("wk", wk), ("wk2", wk2)):
                    t = wpool.tile([128, 8 * DG], F32R, tag=nm, name=nm)
                    nc.sync.dma_start(t[:], par[:].bitcast(F32R))
                    projs.append(t)
                wq_sb, wk_sb, wk2_sb = projs

                plan = [(wq_sb, qT, "bq"), (wk_sb, kT, "bk"), (wk2_sb, k2T, "bk2")]
                for n in range(4):
                    accs = []
                    for c in range(8):
                        xp = xs.tile([128, 512], F32R, tag="xp")
                        nc.sync.dma_start(
                            xp[:], xT[c * 128:(c + 1) * 128,
                                      n * 512:(n + 1) * 512].bitcast(F32R))
                        for pi, (w_sb, dsts, bnm) in enumerate(plan):
                            for p in range(2):
                                if c == 0:
                                    accs.append(pproj.tile([128, 512], F32,
                                                           tag=f"acc{pi}{p}", name=f"acc{pi}{p}"))
                                nc.tensor.matmul(
                                    accs[pi * 2 + p][:],
                                    w_sb[:, c * 256 + p * 128: c * 256 + p * 128 + 128],
                                    xp[:],
                                    start=(c == 0), stop=(c == 7))
                    for pi, (w_sb, dsts, bnm) in enumerate(plan):
                        for p in range(2):
                            nc.vector.tensor_scalar_add(
                                dsts[p][:, n * 512:(n + 1) * 512],
                                accs[pi * 2 + p][:],
                                bias_sb[bnm][:, p:p + 1])

            # pad col for shifted q (col T = col T-1; value discarded)
            for p in range(2):
                nc.vector.tensor_copy(qT[p][:, T:T + 1], qT[p][:, T - 1:T])

            # ---- attention helper ----
            actx = contextlib.ExitStack()
            ps_pool = actx.enter_context(
                tc.tile_pool(name="ps", bufs=2, space="PSUM"))
            py_pool = actx.enter_context(
                tc.tile_pool(name="py", bufs=4, space="PSUM"))
            expp = actx.enter_context(tc.tile_pool(name="expp", bufs=3))

            deferred = []   # (fn) finalize closures, flushed 2 groups late

            def flush_deferred(keep=2):
                while len(deferred) > keep:
                    deferred.pop(0)()

            def attn_pass(h, keysT, statT, qoff, z_write):
                """One causal attention pass for head h.
                keysT: [128, T] pair tile with keys for the pair (row0 offset);
                statT: [128, NT*65] stationary (values+ones);
                qoff: extra column offset into qT (0 or 1);
                z_write(g, py): finalize group's normalized output."""
                p, row0 = h // 2, (h % 2) * 64
                for g in range(NG):
                    flush_deferred(keep=2)
                    py = py_pool.tile([65, 512], F32, tag="py")
                    jlist = list(range(4 * g + 4))
                    for chunk in _chunks(jlist, CHUNK):
                        ps = ps_pool.tile([128, 512 * CHUNK], F32, tag="ps")
                        for idx, j in enumerate(chunk):
                            col0 = max(0, (j - 4 * g) * 128)
                            nc.tensor.matmul(
                                ps[:, idx * 512 + col0:(idx + 1) * 512],
                                keysT[row0:row0 + 64, j * 128:(j + 1) * 128],
                                qT[p][row0:row0 + 64,
                                      qoff + g * 512 + col0: qoff + (g + 1) * 512],
                                start=True, stop=True)
                            if j >= 4 * g:
                                nc.vector.tensor_add(
                                    ps[:, idx * 512 + col0: idx * 512 + col0 + 128],
                                    ps[:, idx * 512 + col0: idx * 512 + col0 + 128],
                                    mask_sb[:])
                        texp = expp.tile([128, 512 * CHUNK], BF16, tag="exp")
                        w = 512 * len(chunk)
                        nc.scalar.activation(texp[:, 0:w], ps[:, 0:w],
                                             mybir.ActivationFunctionType.Exp,
                                             scale=0.125)
                        for idx, j in enumerate(chunk):
                            col0 = max(0, (j - 4 * g) * 128)
                            nc.tensor.matmul(
                                py[:, col0:512],
                                statT[:, j * 65:(j + 1) * 65],
                                texp[:, idx * 512 + col0:(idx + 1) * 512],
                                start=(j == 0), stop=(j == jlist[-1]),
                                skip_group_check=True)
                    deferred.append(lambda g=g, py=py: z_write(g, py))

            # ---- pass 1 (AR) ----
            def run_pass(h, keysT, statT, qoff, fin):
                def zw(g, py):
                    rec = small.tile([1, 512], F32, tag="rec", name="rec")
                    nc.vector.reciprocal(rec[:], py[64:65, :])
                    rbc = small.tile([64, 512], F32, tag="rbc", name="rbc")
                    nc.gpsimd.partition_broadcast(rbc[:], rec[:])
                    fin(g, py, rbc)
                attn_pass(h, keysT, statT, qoff, zw)

            def z1_fin(h):
                p, row0 = h // 2, (h % 2) * 64

                def fin(g, py, rbc):
                    nc.vector.tensor_mul(
                        zT[p][row0:row0 + 64, g * 512:(g + 1) * 512],
                        py[0:64, :], rbc[:])
                return fin

            for h in range(HPC):
                run_pass(h, kT[h // 2], vaug_sb[h], 0, z1_fin(h))

            # ---- e = vshift - y  (into eaug, stationary layout) ----
            flush_deferred(keep=0)
            for h in range(HPC):
                p, row0 = h // 2, (h % 2) * 64
                for j in range(NT):
                    ptr = py_pool.tile([128, 64], F32R, tag="py")
                    nc.tensor.transpose(
                        ptr[:], zT[p][row0:row0 + 64, j * 128:(j + 1) * 128],
                        ident_sb[row0:row0 + 64, :])
                    nc.vector.tensor_sub(
                        eaug_sb[h][:, j * 65: j * 65 + 64],
                        vsh_sb[:, j * DG + h * 64: j * DG + h * 64 + 64],
                        ptr[:])
                ones_ap = eaug_sb[h][:].rearrange(
                    "p (j c) -> p j c", c=65)[:, :, 64:65]
                nc.sync.dma_start(ones_ap, onesp[:])

            # ---- pass 2 (MA) ----
            def z2_fin(h):
                p, row0 = h // 2, (h % 2) * 64

                def fin(g, py, rbc):
                    tmp = small.tile([128, 512], F32, tag="tmp")
                    nc.vector.tensor_mul(tmp[row0:row0 + 64, :], py[0:64, :],
                                         rbc[:])
                    wd = 512 if g < NG - 1 else 511
                    dst = zT[p][row0:row0 + 64, g * 512 + 1: g * 512 + 1 + wd]
                    nc.vector.tensor_add(dst, dst, tmp[row0:row0 + 64, 0:wd])
                return fin

            for h in range(HPC):
                run_pass(h, k2T[h // 2], eaug_sb[h], 1, z2_fin(h))

            # ---- out projection: outT[cb] = sum_cc Wp[cc,cb].T @ zT[cc] ----
            flush_deferred(keep=0)
            actx.close()
            with tc.tile_pool(name="po", bufs=2, space="PSUM") as po, \
                 tc.tile_pool(name="ost", bufs=2) as ost:
                for cb in range(8):
                    acc = po.tile([128, T], F32, tag="po")
                    for cc in range(2):
                        for n in range(4):
                            nc.tensor.matmul(
                                acc[:, n * 512:(n + 1) * 512],
                                wp_sb[:, cc * C + cb * 128: cc * C + cb * 128 + 128],
                                zT[cc][:, n * 512:(n + 1) * 512],
                                start=(cc == 0), stop=(cc == 1))
                    stg = ost.tile([128, T], F32, tag="stg", name="stg")
                    nc.scalar.activation(stg[:], acc[:],
                                         mybir.ActivationFunctionType.Copy)
                    nc.sync.dma_start(outT[cb * 128:(cb + 1) * 128, :], stg[:])

    nc.compile()
    return nc


def _get_built():
    global _BUILT
    if _BUILT is None:
        _BUILT = _build()
    return _BUILT


def _prep_core(x, W_attn, b_attn, W_k2, b_k2, W_proj, core):
    b, hg = core // 4, core % 4
    cs = hg * DG
    xb = np.asarray(x[b], dtype=np.float32)
    xT = np.ascontiguousarray(xb.T)
    xh = xb[:, cs:cs + DG]

    import ml_dtypes
    va = np.empty((HPC, NT, 128, 65), np.float32)
    for h in range(HPC):
        va[h, :, :, :64] = xh[:, h * D:(h + 1) * D].reshape(NT, 128, D)
        va[h, :, :, 64] = 1.0
    vaug = va.transpose(0, 2, 1, 3).reshape(HPC * 128, NT * 65).astype(
        ml_dtypes.bfloat16)

    vs = np.concatenate([xh[1:], np.zeros((1, DG), np.float32)], axis=0)
    vsh = vs.reshape(NT, 128, DG).transpose(1, 0, 2).reshape(128, NT * DG)

    def wslice(Wfull, c0):
        return np.ascontiguousarray(
            Wfull[:, c0:c0 + DG].reshape(8, 128, DG).transpose(1, 0, 2)
            .reshape(128, 8 * DG))

    wq = wslice(W_attn, cs)
    wk = wslice(W_attn, C + cs)
    wk2 = wslice(W_k2, cs)
    wp = np.ascontiguousarray(
        W_proj[cs:cs + DG, :].reshape(2, 128, C).transpose(1, 0, 2)
        .reshape(128, 2 * C))

    bqv = np.ascontiguousarray(b_attn[cs:cs + DG].reshape(2, 128).T)
    bkv = np.ascontiguousarray(b_attn[C + cs:C + cs + DG].reshape(2, 128).T)
    bk2v = np.ascontiguousarray(b_k2[cs:cs + DG].reshape(2, 128).T)

    return dict(xT=xT, vaug=vaug, vsh=vsh, wq=wq, wk=wk, wk2=wk2, wp=wp,
                bq=bqv, bk=bkv, bk2=bk2v)


def kernel(x, W_attn, b_attn, W_k2, b_k2, W_proj, b_proj):
    x = np.asarray(x, np.float32)
    W_attn = np.asarray(W_attn, np.float32)
    b_attn = np.asarray(b_attn, np.float32)
    W_k2 = np.asarray(W_k2, np.float32)
    b_k2 = np.asarray(b_k2, np.float32)
    W_proj = np.asarray(W_proj, np.float32)
    b_proj = np.asarray(b_proj, np.float32)

    mask = np.zeros((128, 128), np.float32)
    for kj in range(1, 128):
        mask[kj, :kj] = -1e5
    ident = np.concatenate([np.eye(64, dtype=np.float32)] * 2, axis=0)

    in_maps = []
    for core in range(8):
        m = _prep_core(x, W_attn, b_attn, W_k2, b_k2, W_proj, core)
        m["mask"] = mask
        m["ident"] = ident
        import ml_dtypes as _md; m["ones16"] = np.ones((128, 16, 1), _md.bfloat16)
        in_maps.append(m)

    nc = _get_built()
    res = run_bass_kernel_spmd(nc, in_maps, list(range(8)))

    out = np.zeros((B, T, C), np.float32)
    for core in range(8):
        out[core // 4] += res.results[core]["outT"].T
    out += 2.0 * b_proj
    return out
